# revision 1
# baseline (speedup 1.0000x reference)
"""BoxFilter kernel for Trainium2 (8 NeuronCores).

Computes out[b,0,i,j] = sum_{c} sum_{|di|<=15} sum_{|dj|<=15} x[b,c,i+di,j+dj]
(edge-clamped 31x31 box filter over the channel-summed image), matching the
reference cumsum + shifted-diff formulation exactly (separable box sums).

Sharding: data-parallel over (batch, H-half) -> 8 shards, no cross-core
communication. Each core receives a host-padded [3, 1056, 2048] slab
(16 halo rows on each side, zero-filled past the global image edges).

Per-core pipeline (all f32):
  1. channel-sum on DVE (2 adds per 128-row tile)
  2. vertical 31-tap box sum via two banded 0/1-matrix fp32 matmuls per
     PSUM bank (bands are compile-time constant inputs)
  3. ACT copies PSUM -> zero-padded SBUF tile
  4. horizontal 31-tap box sum in a single tensor_tensor_scan:
     state_j = state_{j-1} + xp[j] - xp[j-31]
  5. DMA result rows to DRAM
"""

import numpy as np

R = 15
TAP = 2 * R + 1          # 31
B, C, H, W = 4, 3, 2048, 2048
HALF = H // 2            # 1024 output rows per core
S_ROWS = HALF + 32       # 1056 input rows per core (16-row halo each side)
N_CORES = 8
PAD_L = TAP              # left zero pad for the scan (31)
PAD_R = R                # right zero pad (15)
XP_W = PAD_L + W + PAD_R # 2094
SCAN_N = W + R           # 2063 scan steps; out col j = scan[j + R]
P = 128                  # SBUF partitions
N_OUT_TILES = HALF // P  # 8
TAIL_ROWS = S_ROWS - N_OUT_TILES * P  # 32 valid rows in the 9th s-tile
MM_N = 512               # fp32 moving-operand max / one PSUM bank

_CACHE = {}


def _band_matrices():
    # out row i of a 128-row tile needs halo'd input rows r = i+1 .. i+31
    # (r is the row index within the [s_lo; s_hi] 256-row window).
    k = np.arange(P)[:, None]
    i = np.arange(P)[None, :]
    band_a = ((k >= i + 1) & (k <= i + TAP)).astype(np.float32)          # rows in s_lo
    band_b = ((k + P >= i + 1) & (k + P <= i + TAP)).astype(np.float32)  # rows in s_hi
    return band_a, band_b


def _build_kernel(tc, nc, out, xs, band_a_d, band_b_d, mybir, bass):
    from contextlib import ExitStack

    f32 = mybir.dt.float32
    f32r = mybir.dt.float32r
    add = mybir.AluOpType.add
    sub = mybir.AluOpType.subtract

    with ExitStack() as ctx:
        const_pool = ctx.enter_context(tc.tile_pool(name="const", bufs=1))
        s_pool = ctx.enter_context(tc.tile_pool(name="s", bufs=4))
        xp_pool = ctx.enter_context(tc.tile_pool(name="xp", bufs=2))
        box_pool = ctx.enter_context(tc.tile_pool(name="box", bufs=2))
        psum_pool = ctx.enter_context(
            tc.tile_pool(name="psum", bufs=8, space=bass.MemorySpace.PSUM)
        )

        xc_pool = ctx.enter_context(tc.tile_pool(name="xc", bufs=4))

        band_a = const_pool.tile([P, P], f32r)
        band_b = const_pool.tile([P, P], f32r)
        nc.sync.dma_start(band_a[:], band_a_d)
        nc.sync.dma_start(band_b[:], band_b_d)

        def make_s(u):
            rows = P if u < N_OUT_TILES else TAIL_ROWS
            s = s_pool.tile([P, W], f32r)
            if rows < P:
                # rows >= TAIL_ROWS are multiplied by zero band weights but
                # must be finite, and rows 31.. are simply past the image.
                nc.gpsimd.memset(s[:].bitcast(f32), 0.0)
            if u < 2:
                # pipeline-fill fast path: the first matmul needs s_0 AND
                # s_1, so land them ASAP — one 1MB DMA per channel spread
                # over all three DMA lanes (sync HWDGE, scalar HWDGE,
                # gpsimd SWDGE), adds on the fast engine (DVE).
                xc = xc_pool.tile([P, C, W], f32)
                for c, eng in ((0, nc.sync), (1, nc.scalar), (2, nc.gpsimd)):
                    eng.dma_start(
                        xc[:rows, c, :], xs[c, P * u : P * u + rows, :]
                    )
                nc.vector.tensor_add(s[:rows, :], xc[:rows, 0, :], xc[:rows, 1, :])
                nc.vector.tensor_add(s[:rows, :], s[:rows, :], xc[:rows, 2, :])
                return s
            # steady state: one batched DMA for all 3 channels: [rows, 3, W],
            # partition-major, alternating HWDGE rings (sync vs scalar) — a
            # single logical DMA queue tops out well below per-core HBM
            # bandwidth.
            xc = xc_pool.tile([P, C, W], f32)
            dma_eng = nc.sync if u % 2 == 0 else nc.scalar
            dma_eng.dma_start(
                xc[:rows],
                xs[:, P * u : P * u + rows, :].rearrange("c p n -> p c n"),
            )
            # split the 2-input adds between DVE and GpSimd so neither
            # engine becomes the pipeline gate (GpSimd TT is ~2x slower)
            eng = nc.vector if u % 2 == 0 else nc.gpsimd
            eng.tensor_add(s[:rows, :], xc[:rows, 0, :], xc[:rows, 1, :])
            eng.tensor_add(s[:rows, :], s[:rows, :], xc[:rows, 2, :])
            return s

        s_tiles = {0: make_s(0)}
        for t in range(N_OUT_TILES):
            s_tiles[t + 1] = make_s(t + 1)
            s_lo, s_hi = s_tiles.pop(t), s_tiles[t + 1]

            xp = xp_pool.tile([P, XP_W], f32)
            nc.gpsimd.memset(xp[:, 0:PAD_L], 0.0)
            nc.gpsimd.memset(xp[:, PAD_L + W : XP_W], 0.0)

            # all band_a matmuls, then all band_b: minimizes PE weight reloads
            psums = []
            for nb in range(W // MM_N):
                ps = psum_pool.tile([P, MM_N], f32)
                lo_c = s_lo[:, MM_N * nb : MM_N * (nb + 1)]
                nc.tensor.matmul(
                    ps[:], band_a[:], lo_c, start=True, stop=False
                )
                psums.append(ps)
            for nb in range(W // MM_N):
                hi_c = s_hi[:, MM_N * nb : MM_N * (nb + 1)]
                nc.tensor.matmul(
                    psums[nb][:], band_b[:], hi_c,
                    start=False, stop=True,
                )
                nc.scalar.copy(
                    xp[:, PAD_L + MM_N * nb : PAD_L + MM_N * (nb + 1)],
                    psums[nb][:],
                )

            box = box_pool.tile([P, SCAN_N + 1], f32)
            nc.vector.tensor_tensor_scan(
                box[:, 0:SCAN_N],
                xp[:, PAD_L : PAD_L + SCAN_N],
                xp[:, 0:SCAN_N],
                0.0,
                add,
                sub,
            )
            store_eng = nc.scalar if t % 2 == 0 else nc.sync
            store_eng.dma_start(out[P * t : P * (t + 1), :], box[:, R : R + W])


def _get_nc():
    if "nc" in _CACHE:
        return _CACHE["nc"]
    import concourse.bass as bass
    import concourse.tile as tile
    from concourse import bacc, mybir

    nc = bacc.Bacc(
        "TRN2", target_bir_lowering=False, debug=False, num_devices=N_CORES
    )
    xs = nc.dram_tensor("xs", [C, S_ROWS, W], mybir.dt.float32, kind="ExternalInput")
    ba = nc.dram_tensor("band_a", [P, P], mybir.dt.float32r, kind="ExternalInput")
    bb = nc.dram_tensor("band_b", [P, P], mybir.dt.float32r, kind="ExternalInput")
    out = nc.dram_tensor("out", [HALF, W], mybir.dt.float32, kind="ExternalOutput")

    with tile.TileContext(nc) as tc:
        _build_kernel(tc, nc, out.ap(), xs.ap(), ba.ap(), bb.ap(), mybir, bass)
    nc.compile()
    _CACHE["nc"] = nc
    return nc


def _in_maps(x):
    band_a, band_b = _band_matrices()
    maps = []
    for k in range(N_CORES):
        b, half = divmod(k, 2)
        h0 = half * HALF
        lo = h0 - 16  # global row of xs row 0
        g0, g1 = max(lo, 0), min(h0 + HALF + 16, H)
        xs = np.zeros((C, S_ROWS, W), np.float32)
        xs[:, g0 - lo : g1 - lo, :] = x[b, :, g0:g1, :]
        maps.append({"xs": xs, "band_a": band_a, "band_b": band_b})
    return maps


def _run(x, trace=False, tmpdir=None):
    from concourse.bass_utils import run_bass_kernel_spmd

    nc = _get_nc()
    res = run_bass_kernel_spmd(
        nc, _in_maps(x), list(range(N_CORES)), trace=trace, tmpdir=tmpdir
    )
    out = np.empty((B, 1, H, W), np.float32)
    for k in range(N_CORES):
        b, half = divmod(k, 2)
        out[b, 0, half * HALF : (half + 1) * HALF, :] = res.results[k]["out"]
    return out, res


def kernel(x: np.ndarray) -> np.ndarray:
    x = np.ascontiguousarray(x, dtype=np.float32)
    assert x.shape == (B, C, H, W)
    return _run(x)[0]



# revision 3
# speedup vs baseline: 1.4416x; 1.4416x over previous
"""BoxFilter kernel for Trainium2 (8 NeuronCores).

Computes out[b,0,i,j] = sum_{c} sum_{|di|<=15} sum_{|dj|<=15} x[b,c,i+di,j+dj]
(edge-clamped 31x31 box filter over the channel-summed image), matching the
reference cumsum + shifted-diff formulation exactly (separable box sums).

Sharding: data-parallel over (batch, H-half) -> 8 shards, no cross-core
communication. Each core receives a host-padded bf16 [3, 1056, 2048] slab
(16 halo rows on each side, zero-filled past the global image edges).

The problem is HBM-bandwidth-bound; everything on the wire is bf16
(tolerance is 2e-2 relative, bf16 end-to-end lands ~5e-3):
  1. channel-sum is folded into the input DMAs: ch0 lands via HWDGE, ch1/ch2
     are SWDGE accumulate-DMAs (CCE inline adders) -> zero vector-engine work
  2. vertical 31-tap box sum via two banded 0/1 bf16 matmuls per PSUM bank
     (fp32 PSUM accumulate); the 32-row tail tile contracts K=32 only
  3. ACT copies PSUM (f32) -> zero-padded SBUF tile (bf16)
  4. horizontal 31-tap box sum in one tensor_tensor_scan per row tile
     (state_j = state_{j-1} + xp[j] - xp[j-31]; fp32 internal state)
  5. DMA result rows to DRAM as bf16; host upcasts
"""

import numpy as np
import ml_dtypes

R = 15
TAP = 2 * R + 1          # 31
B, C, H, W = 4, 3, 2048, 2048
HALF = H // 2            # 1024 output rows per core
S_ROWS = HALF + 32       # 1056 input rows per core (16-row halo each side)
N_CORES = 8
PAD_L = TAP              # left zero pad for the scan (31)
PAD_R = R                # right zero pad (15)
XP_W = PAD_L + W + PAD_R # 2094
SCAN_N = W + R           # 2063 scan steps; out col j = scan[j + R]
P = 128                  # SBUF partitions
N_OUT_TILES = HALF // P  # 8
TAIL_ROWS = S_ROWS - N_OUT_TILES * P  # 32 valid rows in the 9th s-tile
MM_N = 512               # one PSUM bank (512 fp32)

_CACHE = {}


def _band_matrices():
    # out row i of a 128-row tile needs halo'd input rows r = i+1 .. i+31
    # (r is the row index within the [s_lo; s_hi] 256-row window).
    k = np.arange(P)[:, None]
    i = np.arange(P)[None, :]
    band_a = ((k >= i + 1) & (k <= i + TAP)).astype(ml_dtypes.bfloat16)
    band_b = ((k + P >= i + 1) & (k + P <= i + TAP)).astype(ml_dtypes.bfloat16)
    return band_a, band_b


def _build_kernel(tc, nc, out, xs, band_a_d, band_b_d, mybir, bass):
    from contextlib import ExitStack

    f32 = mybir.dt.float32
    bf16 = mybir.dt.bfloat16
    add = mybir.AluOpType.add
    sub = mybir.AluOpType.subtract

    with ExitStack() as ctx:
        const_pool = ctx.enter_context(tc.tile_pool(name="const", bufs=1))
        s_pool = ctx.enter_context(tc.tile_pool(name="s", bufs=5))
        xp_pool = ctx.enter_context(tc.tile_pool(name="xp", bufs=1))
        box_pool = ctx.enter_context(tc.tile_pool(name="box", bufs=1))
        psum_pool = ctx.enter_context(
            tc.tile_pool(name="psum", bufs=8, space=bass.MemorySpace.PSUM)
        )

        band_a = const_pool.tile([P, P], bf16)
        band_b = const_pool.tile([P, P], bf16)
        nc.sync.dma_start(band_a[:], band_a_d)
        nc.sync.dma_start(band_b[:], band_b_d)

        # persistent double-buffered xp/box tiles: the zero pads of xp are
        # written once and stay valid (each iteration only overwrites the
        # middle [PAD_L, PAD_L+W) region).
        xp_tiles = []
        box_tiles = []
        for i in range(2):
            xp = xp_pool.tile([P, XP_W], bf16, tag=f"xp{i}", name=f"xp{i}")
            nc.gpsimd.memset(xp[:, 0:PAD_L], 0.0)
            nc.gpsimd.memset(xp[:, PAD_L + W : XP_W], 0.0)
            xp_tiles.append(xp)
            box = box_pool.tile([P, SCAN_N + 1], bf16, tag=f"box{i}", name=f"box{i}")
            box_tiles.append(box)

        def make_s(u):
            # channel-sum on the wire: plain bf16 load of channel 0 (HWDGE),
            # then two SWDGE accumulate-DMAs (CCE inline add) for ch 1 and 2.
            rows = P if u < N_OUT_TILES else TAIL_ROWS
            s = s_pool.tile([rows, W], bf16, tag="s")
            eng = nc.sync if u % 2 == 0 else nc.scalar
            eng.dma_start(s[:rows, :], xs[0, P * u : P * u + rows, :])
            for c in (1, 2):
                nc.gpsimd.dma_start(
                    s[:rows, :], xs[c, P * u : P * u + rows, :], accum_op=add
                )
            return s

        s_tiles = {0: make_s(0)}
        for t in range(N_OUT_TILES):
            s_tiles[t + 1] = make_s(t + 1)
            s_lo, s_hi = s_tiles.pop(t), s_tiles[t + 1]
            hi_k = P if t + 1 < N_OUT_TILES else TAIL_ROWS

            xp = xp_tiles[t % 2]
            box = box_tiles[t % 2]

            # all band_a matmuls, then all band_b (PSUM f32 accumulate)
            psums = []
            for nb in range(W // MM_N):
                ps = psum_pool.tile([P, MM_N], f32)
                lo_c = s_lo[:, MM_N * nb : MM_N * (nb + 1)]
                nc.tensor.matmul(ps[:], band_a[:], lo_c, start=True, stop=False)
                psums.append(ps)
            for nb in range(W // MM_N):
                hi_c = s_hi[:hi_k, MM_N * nb : MM_N * (nb + 1)]
                nc.tensor.matmul(
                    psums[nb][:], band_b[:hi_k, :], hi_c,
                    start=False, stop=True,
                )
                nc.scalar.copy(
                    xp[:, PAD_L + MM_N * nb : PAD_L + MM_N * (nb + 1)],
                    psums[nb][:],
                )

            nc.vector.tensor_tensor_scan(
                box[:, 0:SCAN_N],
                xp[:, PAD_L : PAD_L + SCAN_N],
                xp[:, 0:SCAN_N],
                0.0,
                add,
                sub,
            )
            store_eng = nc.scalar if t % 2 == 0 else nc.sync
            store_eng.dma_start(out[P * t : P * (t + 1), :], box[:, R : R + W])


def _get_nc():
    if "nc" in _CACHE:
        return _CACHE["nc"]
    import concourse.bass as bass
    import concourse.tile as tile
    from concourse import bacc, mybir

    nc = bacc.Bacc(
        "TRN2", target_bir_lowering=False, debug=False, num_devices=N_CORES
    )
    xs = nc.dram_tensor("xs", [C, S_ROWS, W], mybir.dt.bfloat16, kind="ExternalInput")
    ba = nc.dram_tensor("band_a", [P, P], mybir.dt.bfloat16, kind="ExternalInput")
    bb = nc.dram_tensor("band_b", [P, P], mybir.dt.bfloat16, kind="ExternalInput")
    out = nc.dram_tensor("out", [HALF, W], mybir.dt.bfloat16, kind="ExternalOutput")

    with tile.TileContext(nc) as tc:
        _build_kernel(tc, nc, out.ap(), xs.ap(), ba.ap(), bb.ap(), mybir, bass)
    nc.compile()
    _CACHE["nc"] = nc
    return nc


def _in_maps(x):
    band_a, band_b = _band_matrices()
    xb = x.astype(ml_dtypes.bfloat16)
    maps = []
    for k in range(N_CORES):
        b, half = divmod(k, 2)
        h0 = half * HALF
        lo = h0 - 16  # global row of xs row 0
        g0, g1 = max(lo, 0), min(h0 + HALF + 16, H)
        xs = np.zeros((C, S_ROWS, W), ml_dtypes.bfloat16)
        xs[:, g0 - lo : g1 - lo, :] = xb[b, :, g0:g1, :]
        maps.append({"xs": xs, "band_a": band_a, "band_b": band_b})
    return maps


def _run(x, trace=False, tmpdir=None):
    from concourse.bass_utils import run_bass_kernel_spmd

    nc = _get_nc()
    res = run_bass_kernel_spmd(
        nc, _in_maps(x), list(range(N_CORES)), trace=trace, tmpdir=tmpdir
    )
    out = np.empty((B, 1, H, W), np.float32)
    for k in range(N_CORES):
        b, half = divmod(k, 2)
        out[b, 0, half * HALF : (half + 1) * HALF, :] = np.asarray(
            res.results[k]["out"]
        ).astype(np.float32)
    return out, res


def kernel(x: np.ndarray) -> np.ndarray:
    x = np.ascontiguousarray(x, dtype=np.float32)
    assert x.shape == (B, C, H, W)
    return _run(x)[0]


# revision 4
# speedup vs baseline: 1.6068x; 1.1146x over previous
"""BoxFilter kernel for Trainium2 (8 NeuronCores).

Computes out[b,0,i,j] = sum_{c} sum_{|di|<=15} sum_{|dj|<=15} x[b,c,i+di,j+dj]
(edge-clamped 31x31 box filter over the channel-summed image), matching the
reference cumsum + shifted-diff formulation exactly (separable box sums).

Sharding: data-parallel over (batch, H-half) -> 8 shards, no cross-core
communication. Each core receives a host-padded bf16 [3, 1056, 2048] slab
(16 halo rows on each side, zero-filled past the global image edges).

The problem is HBM-bandwidth-bound; everything on the wire is bf16
(tolerance is 2e-2 relative, bf16 end-to-end lands ~5e-3):
  1. one batched HWDGE DMA per 128-row tile: xc[p, c, n] (1.5 MB)
  2. partial channel sum s01 = ch0 + ch1 (one bf16 DVE add, 2x mode;
     a few tiles' adds go to GpSimd to keep DVE under the DMA roofline)
  3. vertical 31-tap box sum on PE: per PSUM bank, 4 banded bf16 matmuls
     accumulate band_a.T @ s01_lo + band_a.T @ x2_lo + band_b.T @ s01_hi
     + band_b.T @ x2_hi (ch2 is folded into the matmul: PE is far from its
     roofline, the vector engines are not). The 32-row tail contracts K=32.
  4. ACT copies PSUM (f32) -> zero-padded SBUF tile (bf16)
  5. horizontal 31-tap box sum in one tensor_tensor_scan per row tile
     (state_j = state_{j-1} + xp[j] - xp[j-31]; fp32 internal state)
  6. DMA result rows to DRAM as bf16; host upcasts
"""

import numpy as np
import ml_dtypes

R = 15
TAP = 2 * R + 1          # 31
B, C, H, W = 4, 3, 2048, 2048
HALF = H // 2            # 1024 output rows per core
S_ROWS = HALF + 32       # 1056 input rows per core (16-row halo each side)
N_CORES = 8
PAD_L = TAP              # left zero pad for the scan (31)
PAD_R = R                # right zero pad (15)
XP_W = PAD_L + W + PAD_R # 2094
SCAN_N = W + R           # 2063 scan steps; out col j = scan[j + R]
P = 128                  # SBUF partitions
N_OUT_TILES = HALF // P  # 8
TAIL_ROWS = S_ROWS - N_OUT_TILES * P  # 32 valid rows in the 9th s-tile
MM_N = 512               # one PSUM bank (512 fp32)

GPSIMD_ADD_TILES = (2, 4, 6)  # mid-stream tiles whose s01 add runs on GpSimd

_CACHE = {}


def _band_matrices():
    # out row i of a 128-row tile needs halo'd input rows r = i+1 .. i+31
    # (r is the row index within the [s_lo; s_hi] 256-row window).
    k = np.arange(P)[:, None]
    i = np.arange(P)[None, :]
    band_a = ((k >= i + 1) & (k <= i + TAP)).astype(ml_dtypes.bfloat16)
    band_b = ((k + P >= i + 1) & (k + P <= i + TAP)).astype(ml_dtypes.bfloat16)
    return band_a, band_b


def _build_kernel(tc, nc, out, xs, band_a_d, band_b_d, mybir, bass):
    from contextlib import ExitStack

    f32 = mybir.dt.float32
    bf16 = mybir.dt.bfloat16
    add = mybir.AluOpType.add
    sub = mybir.AluOpType.subtract

    with ExitStack() as ctx:
        const_pool = ctx.enter_context(tc.tile_pool(name="const", bufs=1))
        xc_pool = ctx.enter_context(tc.tile_pool(name="xc", bufs=5))
        s_pool = ctx.enter_context(tc.tile_pool(name="s", bufs=4))
        xp_pool = ctx.enter_context(tc.tile_pool(name="xp", bufs=1))
        box_pool = ctx.enter_context(tc.tile_pool(name="box", bufs=1))
        psum_pool = ctx.enter_context(
            tc.tile_pool(name="psum", bufs=8, space=bass.MemorySpace.PSUM)
        )

        band_a = const_pool.tile([P, P], bf16)
        band_b = const_pool.tile([P, P], bf16)
        nc.sync.dma_start(band_a[:], band_a_d)
        nc.sync.dma_start(band_b[:], band_b_d)

        # persistent double-buffered xp/box tiles: the zero pads of xp are
        # written once and stay valid (each iteration only overwrites the
        # middle [PAD_L, PAD_L+W) region).
        xp_tiles = []
        box_tiles = []
        for i in range(2):
            xp = xp_pool.tile([P, XP_W], bf16, tag=f"xp{i}", name=f"xp{i}")
            nc.gpsimd.memset(xp[:, 0:PAD_L], 0.0)
            nc.gpsimd.memset(xp[:, PAD_L + W : XP_W], 0.0)
            xp_tiles.append(xp)
            box = box_pool.tile([P, SCAN_N + 1], bf16, tag=f"box{i}", name=f"box{i}")
            box_tiles.append(box)

        def make_s(u):
            # one batched DMA for all 3 channels, then s01 = ch0 + ch1
            # (ch2 is consumed directly by the matmuls)
            rows = P if u < N_OUT_TILES else TAIL_ROWS
            xc = xc_pool.tile([rows, C, W], bf16, tag="xc")
            eng = nc.sync if u % 2 == 0 else nc.scalar
            eng.dma_start(
                xc[:rows],
                xs[:, P * u : P * u + rows, :].rearrange("c p n -> p c n"),
            )
            s01 = s_pool.tile([rows, W], bf16, tag="s01", name=f"s01_{u}")
            add_eng = nc.gpsimd if u in GPSIMD_ADD_TILES else nc.vector
            add_eng.tensor_add(s01[:rows, :], xc[:rows, 0, :], xc[:rows, 1, :])
            return xc, s01

        s_tiles = {0: make_s(0)}
        for t in range(N_OUT_TILES):
            s_tiles[t + 1] = make_s(t + 1)
            (xc_lo, s_lo), (xc_hi, s_hi) = s_tiles.pop(t), s_tiles[t + 1]
            hi_k = P if t + 1 < N_OUT_TILES else TAIL_ROWS

            xp = xp_tiles[t % 2]
            box = box_tiles[t % 2]

            # vertical box sums in PSUM (f32): 4 bf16 matmuls per bank
            psums = []
            for nb in range(W // MM_N):
                cs = slice(MM_N * nb, MM_N * (nb + 1))
                ps = psum_pool.tile([P, MM_N], f32, tag="ps")
                nc.tensor.matmul(ps[:], band_a[:], s_lo[:, cs], start=True, stop=False)
                nc.tensor.matmul(
                    ps[:], band_a[:], xc_lo[:, 2, cs], start=False, stop=False
                )
                psums.append(ps)
            for nb in range(W // MM_N):
                cs = slice(MM_N * nb, MM_N * (nb + 1))
                nc.tensor.matmul(
                    psums[nb][:], band_b[:hi_k, :], s_hi[:hi_k, cs],
                    start=False, stop=False,
                )
                nc.tensor.matmul(
                    psums[nb][:], band_b[:hi_k, :], xc_hi[:hi_k, 2, cs],
                    start=False, stop=True,
                )
                nc.scalar.copy(xp[:, PAD_L + MM_N * nb : PAD_L + MM_N * (nb + 1)],
                               psums[nb][:])

            nc.vector.tensor_tensor_scan(
                box[:, 0:SCAN_N],
                xp[:, PAD_L : PAD_L + SCAN_N],
                xp[:, 0:SCAN_N],
                0.0,
                add,
                sub,
            )
            store_eng = nc.scalar if t % 2 == 0 else nc.sync
            store_eng.dma_start(out[P * t : P * (t + 1), :], box[:, R : R + W])


def _get_nc():
    if "nc" in _CACHE:
        return _CACHE["nc"]
    import concourse.bass as bass
    import concourse.tile as tile
    from concourse import bacc, mybir

    nc = bacc.Bacc(
        "TRN2", target_bir_lowering=False, debug=False, num_devices=N_CORES
    )
    xs = nc.dram_tensor("xs", [C, S_ROWS, W], mybir.dt.bfloat16, kind="ExternalInput")
    ba = nc.dram_tensor("band_a", [P, P], mybir.dt.bfloat16, kind="ExternalInput")
    bb = nc.dram_tensor("band_b", [P, P], mybir.dt.bfloat16, kind="ExternalInput")
    out = nc.dram_tensor("out", [HALF, W], mybir.dt.bfloat16, kind="ExternalOutput")

    with tile.TileContext(nc) as tc:
        _build_kernel(tc, nc, out.ap(), xs.ap(), ba.ap(), bb.ap(), mybir, bass)
    nc.compile()
    _CACHE["nc"] = nc
    return nc


def _in_maps(x):
    band_a, band_b = _band_matrices()
    xb = x.astype(ml_dtypes.bfloat16)
    maps = []
    for k in range(N_CORES):
        b, half = divmod(k, 2)
        h0 = half * HALF
        lo = h0 - 16  # global row of xs row 0
        g0, g1 = max(lo, 0), min(h0 + HALF + 16, H)
        xs = np.zeros((C, S_ROWS, W), ml_dtypes.bfloat16)
        xs[:, g0 - lo : g1 - lo, :] = xb[b, :, g0:g1, :]
        maps.append({"xs": xs, "band_a": band_a, "band_b": band_b})
    return maps


def _run(x, trace=False, tmpdir=None):
    from concourse.bass_utils import run_bass_kernel_spmd

    nc = _get_nc()
    res = run_bass_kernel_spmd(
        nc, _in_maps(x), list(range(N_CORES)), trace=trace, tmpdir=tmpdir
    )
    out = np.empty((B, 1, H, W), np.float32)
    for k in range(N_CORES):
        b, half = divmod(k, 2)
        out[b, 0, half * HALF : (half + 1) * HALF, :] = np.asarray(
            res.results[k]["out"]
        ).astype(np.float32)
    return out, res


def kernel(x: np.ndarray) -> np.ndarray:
    x = np.ascontiguousarray(x, dtype=np.float32)
    assert x.shape == (B, C, H, W)
    return _run(x)[0]
